# revision 15
# baseline (speedup 1.0000x reference)
"""Trainium2 Bass SPMD kernel for AlignUniformLoss over 8 NeuronCores.

Math (matches reference):
    a_n, b_n = row-wise L2 normalize of embeddings_a / embeddings_b
    align    = mean_i ||a_n_i - b_n_i||^2 = mean_i (2 - 2<a_n_i, b_n_i>)
    x        = rows of a_n and b_n stacked (any row permutation is fine --
               the uniformity term is permutation invariant; we interleave
               so each core owns matching a/b row blocks)
    e_ij     = exp(-2 * d_ij^2) = exp(4 <x_i, x_j> - 4)   (unit rows)
    uniform  = (sum_ij e_ij - n) / (n * (n - 1)),  n = 2B
    loss     = align + uniform

Sharding: core k owns x rows [k*n/8, (k+1)*n/8) (= pairs [k*B/8,(k+1)*B/8) of
a and b). Each core normalizes its rows (fp32, Newton-refined rsqrt), casts
to bf16, transposes via the PE, AllGathers the d-major chunks, then computes
its [n/8, n] slab of the gram matrix with bf16 matmuls into PSUM. One scalar
engine activation per PSUM supertile evaluates exp(4g-4) with a fused
per-partition running sum (accum_out). The align dot products come from the
core's own normalized rows. Host glue sums 8x[128,2] partials.
"""

import numpy as np

_PROGRAMS = {}

FULL_CFG = dict(B=4096, D=256, cores=8)


def build_program(cfg):
    import concourse.bass as bass
    import concourse.tile as tile
    from concourse import bacc, mybir
    import ml_dtypes

    f32 = mybir.dt.float32
    bf16 = mybir.dt.bfloat16
    FT = mybir.ActivationFunctionType
    OP = mybir.AluOpType
    AX = mybir.AxisListType

    B, D, C = cfg["B"], cfg["D"], cfg["cores"]
    n = 2 * B
    chunk = n // C          # x rows per core
    P = 128
    T = chunk // P          # 128-row tiles per core
    TH = T // 2             # a-rows in tiles [0,TH), b-rows in [TH,2TH)
    KT = D // P             # contraction tiles
    TS = min(2048, n)       # PSUM supertile free size (<=4 banks)
    NS = n // TS            # supertiles per row block
    MMN = min(512, chunk)   # matmul free dim
    assert chunk % P == 0 and T % 2 == 0 and D % P == 0
    assert n % TS == 0 and TS % MMN == 0 and chunk % MMN == 0 and TS <= 4096

    nc = bacc.Bacc(
        "TRN2", target_bir_lowering=False, debug=cfg.get("debug", False),
        num_devices=C,
    )

    rows_d = nc.dram_tensor("rows", [chunk, D], f32, kind="ExternalInput")
    stats_d = nc.dram_tensor("stats", [P, 2], f32, kind="ExternalOutput")
    ident_d = nc.inline_tensor(np.eye(P, dtype=ml_dtypes.bfloat16), name="ident")

    with tile.TileContext(nc) as tc:
        with (
            tc.tile_pool(name="const", bufs=1) as constp,
            tc.tile_pool(name="io", bufs=1) as iop,
            tc.tile_pool(name="small", bufs=1) as smallp,
            tc.tile_pool(name="tmp", bufs=2) as tmpp,
            tc.tile_pool(name="dram", bufs=1, space="DRAM") as dramp,
        ):
            ident = constp.tile([P, P], bf16, tag="ident")
            nc.sync.dma_start(ident[:], ident_d.ap())
            neg4 = constp.tile([P, 1], f32, tag="neg4")
            nc.gpsimd.memset(neg4[:], -4.0)

            # ---- load own rows ----
            raw = iop.tile([P, T, D], f32, tag="raw")
            nc.sync.dma_start(
                raw[:], rows_d.ap().rearrange("(t p) d -> p t d", p=P)
            )

            # ---- row norms ----
            sq = iop.tile([P, T, D], f32, tag="sq")
            nc.vector.tensor_mul(sq[:], raw[:], raw[:])
            ss = smallp.tile([P, T], f32, tag="ss")
            nc.vector.tensor_reduce(ss[:], sq[:], axis=AX.X, op=OP.add)

            # invnorm = rsqrt(ss); ACT Sqrt + DVE reciprocal seed, then two
            # Newton steps (ACT sqrt has a loose ULP budget).
            r0 = smallp.tile([P, T], f32, tag="r0")
            nc.scalar.activation(r0[:], ss[:], FT.Sqrt)
            y = smallp.tile([P, T], f32, tag="y")
            nc.vector.reciprocal(y[:], r0[:])
            for it in range(2):
                t1 = smallp.tile([P, T], f32, tag=f"nt1_{it}")
                nc.vector.tensor_mul(t1[:], y[:], y[:])
                t2 = smallp.tile([P, T], f32, tag=f"nt2_{it}")
                nc.vector.tensor_mul(t2[:], t1[:], ss[:])
                t3 = smallp.tile([P, T], f32, tag=f"nt3_{it}")
                nc.vector.tensor_scalar(t3[:], t2[:], -0.5, 1.5, OP.mult, OP.add)
                y2 = smallp.tile([P, T], f32, tag=f"ny_{it}")
                nc.vector.tensor_mul(y2[:], y[:], t3[:])
                y = y2

            # ---- normalize + cast to bf16 ----
            normb = iop.tile([P, T, D], bf16, tag="normb")
            for t in range(T):
                nc.vector.tensor_scalar_mul(
                    normb[:, t, :], raw[:, t, :], y[:, t:t + 1]
                )

            # ---- align dots: <a_i, b_i> for the core's pairs ----
            dots = smallp.tile([P, TH], f32, tag="dots")
            for t in range(TH):
                j1 = tmpp.tile([P, D], bf16, tag="sttjunk")
                nc.vector.scalar_tensor_tensor(
                    j1[:], normb[:, t, :], 1.0, normb[:, t + TH, :],
                    OP.mult, OP.mult, accum_out=dots[:, t:t + 1],
                )

            # ---- transpose own chunk to d-major (xT[kt] is [P, chunk]) ----
            xT = iop.tile([P, KT, chunk], bf16, tag="xT")
            psp = tc.alloc_tile_pool(name="ps", bufs=2, space="PSUM")
            # One bf16 PSUM tile holds all transpose outputs; shares the
            # "ps" tag with main-loop tiles so PSUM reuse hazards stay on a
            # single already-observed semaphore.
            tp_all = psp.tile([P, T * KT * P], bf16, tag="ps")
            for t in range(T):
                for kt in range(KT):
                    o = (t * KT + kt) * P
                    nc.tensor.transpose(
                        tp_all[:, o:o + P], normb[:, t, kt * P:(kt + 1) * P],
                        ident[:],
                    )
                    nc.vector.tensor_copy(
                        xT[:, kt, t * P:(t + 1) * P], tp_all[:, o:o + P]
                    )

            # ---- AllGather the normalized, transposed chunks ----
            cc_in = dramp.tile([KT, P, chunk], bf16, tag="cc_in")
            cc_out = dramp.tile(
                [C, KT, P, chunk], bf16, tag="cc_out", addr_space="Shared"
            )
            # One DMA (one wait on the collective): dst iterated p-major to
            # match the SBUF source layout.
            nc.sync.dma_start(
                cc_in[:].rearrange("kt p c -> p kt c"), xT[:]
            )
            nc.gpsimd.collective_compute(
                "AllGather",
                OP.bypass,
                replica_groups=[list(range(C))],
                ins=[cc_in[:].opt()],
                outs=[cc_out[:].opt()],
            )
            rhs = [
                [
                    iop.tile(
                        [P, chunk], bf16, tag=f"rhs{kt}_{r}", name=f"rhs{kt}_{r}"
                    )
                    for r in range(C)
                ]
                for kt in range(KT)
            ]
            for r in range(C):
                for kt in range(KT):
                    nc.sync.dma_start(rhs[kt][r][:], cc_out[r, kt])

            # ---- main loop: gram supertiles -> exp -> fused row sums ----
            accs = smallp.tile([P, T * NS], f32, tag="accs")
            for m in range(T):
                for n4 in range(NS):
                    ps = psp.tile([P, TS], f32, tag="ps", name=f"ps_{m}_{n4}")
                    for s in range(TS // MMN):
                        col = n4 * TS + s * MMN
                        rch, off = divmod(col, chunk)
                        for kt in range(KT):
                            nc.tensor.matmul(
                                ps[:, s * MMN:(s + 1) * MMN],
                                xT[:, kt, m * P:(m + 1) * P],
                                rhs[kt][rch][:, off:off + MMN],
                                start=(kt == 0),
                                stop=(kt == KT - 1),
                            )
                    jt = tmpp.tile([P, TS], bf16, tag="actjunk")
                    idx = m * NS + n4
                    nc.scalar.activation(
                        jt[:], ps[:], FT.Exp, bias=neg4[:], scale=4.0,
                        accum_out=accs[:, idx:idx + 1],
                    )
            psp.release()

            # ---- fold partials and store ----
            stats = smallp.tile([P, 2], f32, tag="stats")
            nc.vector.tensor_reduce(stats[:, 0:1], dots[:], axis=AX.X, op=OP.add)
            nc.vector.tensor_reduce(stats[:, 1:2], accs[:], axis=AX.X, op=OP.add)
            nc.sync.dma_start(stats_d.ap(), stats[:])

    nc.compile()
    return nc


def get_program(cfg):
    key = tuple(sorted(cfg.items()))
    if key not in _PROGRAMS:
        _PROGRAMS[key] = build_program(cfg)
    return _PROGRAMS[key]


def shard_inputs(a, b, cfg):
    B, C = cfg["B"], cfg["cores"]
    pc = B // C
    a = np.ascontiguousarray(np.asarray(a, dtype=np.float32))
    b = np.ascontiguousarray(np.asarray(b, dtype=np.float32))
    maps = []
    for k in range(C):
        rows = np.concatenate([a[k * pc:(k + 1) * pc], b[k * pc:(k + 1) * pc]], 0)
        maps.append({"rows": np.ascontiguousarray(rows)})
    return maps


def combine(per_core_stats, cfg):
    B = cfg["B"]
    n = 2 * B
    d_sum = 0.0
    s_sum = 0.0
    for st in per_core_stats:
        st = np.asarray(st, dtype=np.float64)
        d_sum += st[:, 0].sum()
        s_sum += st[:, 1].sum()
    align = 2.0 - (2.0 / B) * d_sum
    uniform = (s_sum - n) / (n * (n - 1.0))
    return np.float32(align + uniform)


def run_spmd(a, b, cfg, trace=False):
    from concourse.bass_utils import run_bass_kernel_spmd

    nc = get_program(cfg)
    in_maps = shard_inputs(a, b, cfg)
    res = run_bass_kernel_spmd(
        nc, in_maps, core_ids=list(range(cfg["cores"])), trace=trace
    )
    out = combine([r["stats"] for r in res.results], cfg)
    return out, res


def kernel(embeddings_a, embeddings_b):
    out, _ = run_spmd(embeddings_a, embeddings_b, FULL_CFG, trace=False)
    return np.asarray(out)


# revision 16
# speedup vs baseline: 1.1677x; 1.1677x over previous
"""Trainium2 Bass SPMD kernel for AlignUniformLoss over 8 NeuronCores.

Math (matches reference):
    a_n, b_n = row-wise L2 normalize of embeddings_a / embeddings_b
    align    = mean_i ||a_n_i - b_n_i||^2 = mean_i (2 - 2<a_n_i, b_n_i>)
    x        = rows of a_n and b_n stacked (any row permutation works --
               the uniformity term is permutation invariant; we interleave
               so each core owns matching a/b row blocks)
    e_ij     = exp(-2 * d_ij^2) = exp(4 <x_i, x_j> - 4)   (unit rows)
    uniform  = (sum_ij e_ij - n) / (n * (n - 1)),  n = 2B
    loss     = align + uniform

Sharding: core k owns x rows [k*n/8, (k+1)*n/8). It normalizes its rows
(rsqrt seeded by exp(-0.5*ln(ss)) -- keeps ACT on one table set -- plus two
Newton steps), transposes them via the PE, and AllGathers the d-major
chunks as x8-scaled fp8 (wire bytes halved; the x8 avoids fp8 denormals;
the gram exponent scale absorbs the factor). The AllGather is split into G
pieces pipelined against the main loop: gram supertiles of piece g overlap
the transfer of piece g+1. One scalar-engine activation per [128, TS] PSUM
supertile evaluates exp((4/s^2) g - 4) with a fused per-partition sum
(accum_out). Align dots come from the core's own bf16 normalized rows.
Host glue sums 8 x [128, 2] partials.
"""

import numpy as np

_PROGRAMS = {}

FULL_CFG = dict(B=4096, D=256, cores=8, G=2, wire="fp8")


def build_program(cfg):
    import concourse.bass as bass
    import concourse.tile as tile
    from concourse import bacc, mybir
    import ml_dtypes

    f32 = mybir.dt.float32
    bf16 = mybir.dt.bfloat16
    FT = mybir.ActivationFunctionType
    OP = mybir.AluOpType
    AX = mybir.AxisListType

    B, D, C = cfg["B"], cfg["D"], cfg["cores"]
    G = cfg.get("G", 2)
    wire = cfg.get("wire", "fp8")
    wdt = mybir.dt.float8e4 if wire == "fp8" else bf16
    WS = 8.0 if wire == "fp8" else 1.0   # wire scale
    n = 2 * B
    chunk = n // C            # x rows per core
    piece = chunk // G        # rows per AG piece
    P = 128
    T = chunk // P            # 128-row tiles per core
    TH = T // 2               # a-rows in tiles [0,TH), b-rows in [TH,2TH)
    KT = D // P               # contraction tiles
    TS = min(2048, C * piece)  # PSUM supertile free size (<=4 banks)
    NQ = (C * piece) // TS    # supertiles per (row block, piece)
    MMN = min(512, piece)     # matmul free dim
    SPS = TS // MMN           # matmul slices per supertile
    assert chunk % (P * G) == 0 and T % 2 == 0 and D % P == 0
    assert (C * piece) % TS == 0 and TS % MMN == 0 and piece % MMN == 0

    nc = bacc.Bacc(
        "TRN2", target_bir_lowering=False, debug=cfg.get("debug", False),
        num_devices=C,
    )

    rows_d = nc.dram_tensor("rows", [chunk, D], f32, kind="ExternalInput")
    stats_d = nc.dram_tensor("stats", [P, 2], f32, kind="ExternalOutput")
    ident_d = nc.inline_tensor(np.eye(P, dtype=ml_dtypes.bfloat16), name="ident")

    with tile.TileContext(nc) as tc:
        with (
            tc.tile_pool(name="const", bufs=1) as constp,
            tc.tile_pool(name="io", bufs=1) as iop,
            tc.tile_pool(name="small", bufs=1) as smallp,
            tc.tile_pool(name="tmp", bufs=2) as tmpp,
            tc.tile_pool(name="dram", bufs=1, space="DRAM") as dramp,
        ):
            ident = constp.tile([P, P], bf16, tag="ident")
            nc.sync.dma_start(ident[:], ident_d.ap())
            neg4 = constp.tile([P, 1], f32, tag="neg4")
            nc.gpsimd.memset(neg4[:], -4.0)

            # ---- load own rows ----
            raw = iop.tile([P, T, D], f32, tag="raw")
            nc.sync.dma_start(
                raw[:], rows_d.ap().rearrange("(t p) d -> p t d", p=P)
            )

            # ---- row norms: ss[t] = sum_d raw^2 (fused square+rowsum) ----
            ss = smallp.tile([P, T], f32, tag="ss")
            for t in range(T):
                sqj = tmpp.tile([P, D], f32, tag="sqjunk")
                nc.vector.scalar_tensor_tensor(
                    sqj[:], raw[:, t, :], 1.0, raw[:, t, :],
                    OP.mult, OP.mult, accum_out=ss[:, t:t + 1],
                )

            # invnorm = rsqrt(ss): seed exp(-0.5 ln ss) (one ACT table set
            # for the whole kernel), then two Newton steps on the DVE.
            lns = smallp.tile([P, T], f32, tag="lns")
            nc.scalar.activation(lns[:], ss[:], FT.Ln)
            y = smallp.tile([P, T], f32, tag="y")
            nc.scalar.activation(y[:], lns[:], FT.Exp, scale=-0.5)
            for it in range(2):
                t1 = smallp.tile([P, T], f32, tag=f"nt1_{it}")
                nc.vector.tensor_mul(t1[:], y[:], y[:])
                t2 = smallp.tile([P, T], f32, tag=f"nt2_{it}")
                nc.vector.tensor_mul(t2[:], t1[:], ss[:])
                t3 = smallp.tile([P, T], f32, tag=f"nt3_{it}")
                nc.vector.tensor_scalar(t3[:], t2[:], -0.5, 1.5, OP.mult, OP.add)
                y2 = smallp.tile([P, T], f32, tag=f"ny_{it}")
                nc.vector.tensor_mul(y2[:], y[:], t3[:])
                y = y2

            # ---- normalize + cast to bf16 ----
            normb = iop.tile([P, T, D], bf16, tag="normb")
            for t in range(T):
                nc.vector.tensor_scalar_mul(
                    normb[:, t, :], raw[:, t, :], y[:, t:t + 1]
                )

            # ---- transpose own chunk to d-major, scale into wire dtype ----
            xT = iop.tile([P, KT, chunk], wdt, tag="xT")
            psp = tc.alloc_tile_pool(name="ps", bufs=2, space="PSUM")
            tp_all = psp.tile([P, T * KT * P], bf16, tag="ps")
            for t in range(T):
                for kt in range(KT):
                    o = (t * KT + kt) * P
                    nc.tensor.transpose(
                        tp_all[:, o:o + P], normb[:, t, kt * P:(kt + 1) * P],
                        ident[:],
                    )
                    nc.vector.tensor_scalar_mul(
                        xT[:, kt, t * P:(t + 1) * P], tp_all[:, o:o + P], WS
                    )

            # ---- align dots: <a_i, b_i> for the core's pairs ----
            dots = smallp.tile([P, TH], f32, tag="dots")
            for t in range(TH):
                j1 = tmpp.tile([P, D], bf16, tag="sttjunk")
                nc.vector.scalar_tensor_tensor(
                    j1[:], normb[:, t, :], 1.0, normb[:, t + TH, :],
                    OP.mult, OP.mult, accum_out=dots[:, t:t + 1],
                )

            # ---- pipelined AllGather of the wire chunks, G pieces ----
            cc_in = []
            cc_out = []
            for g in range(G):
                ci = dramp.tile([KT, P, piece], wdt, tag=f"cc_in{g}",
                                name=f"cc_in{g}")
                co = dramp.tile([C, KT, P, piece], wdt, tag=f"cc_out{g}",
                                name=f"cc_out{g}", addr_space="Shared")
                cc_in.append(ci)
                cc_out.append(co)
                nc.sync.dma_start(
                    ci[:].rearrange("kt p c -> p kt c"),
                    xT[:, :, g * piece:(g + 1) * piece],
                )
                nc.gpsimd.collective_compute(
                    "AllGather",
                    OP.bypass,
                    replica_groups=[list(range(C))],
                    ins=[ci[:].opt()],
                    outs=[co[:].opt()],
                )
            rhs = [
                [
                    [
                        iop.tile([P, piece], wdt, tag=f"rhs{g}_{kt}_{r}",
                                 name=f"rhs{g}_{kt}_{r}")
                        for r in range(C)
                    ]
                    for kt in range(KT)
                ]
                for g in range(G)
            ]
            for g in range(G):
                for r in range(C):
                    for kt in range(KT):
                        nc.sync.dma_start(rhs[g][kt][r][:], cc_out[g][r, kt])

            # ---- main loop: gram supertiles -> exp -> fused row sums ----
            # cols of phase g = every rank's piece-g rows; full coverage
            # over g/q/m, each (i, j) exactly once.
            accs = smallp.tile([P, G * T * NQ], f32, tag="accs")
            idx = 0
            for g in range(G):
                for m in range(T):
                    for q in range(NQ):
                        ps = psp.tile([P, TS], f32, tag="ps",
                                      name=f"ps_{g}_{m}_{q}")
                        for s in range(SPS):
                            r, off = divmod(q * TS + s * MMN, piece)
                            for kt in range(KT):
                                nc.tensor.matmul(
                                    ps[:, s * MMN:(s + 1) * MMN],
                                    xT[:, kt, m * P:(m + 1) * P],
                                    rhs[g][kt][r][:, off:off + MMN],
                                    start=(kt == 0),
                                    stop=(kt == KT - 1),
                                )
                        jt = tmpp.tile([P, TS], bf16, tag="actjunk")
                        nc.scalar.activation(
                            jt[:], ps[:], FT.Exp, bias=neg4[:],
                            scale=4.0 / (WS * WS),
                            accum_out=accs[:, idx:idx + 1],
                        )
                        idx += 1
            psp.release()

            # ---- fold partials and store ----
            stats = smallp.tile([P, 2], f32, tag="stats")
            nc.vector.tensor_reduce(stats[:, 0:1], dots[:], axis=AX.X, op=OP.add)
            nc.vector.tensor_reduce(stats[:, 1:2], accs[:], axis=AX.X, op=OP.add)
            nc.sync.dma_start(stats_d.ap(), stats[:])

    nc.compile()
    return nc


def get_program(cfg):
    key = tuple(sorted(cfg.items()))
    if key not in _PROGRAMS:
        _PROGRAMS[key] = build_program(cfg)
    return _PROGRAMS[key]


def shard_inputs(a, b, cfg):
    B, C = cfg["B"], cfg["cores"]
    pc = B // C
    a = np.ascontiguousarray(np.asarray(a, dtype=np.float32))
    b = np.ascontiguousarray(np.asarray(b, dtype=np.float32))
    maps = []
    for k in range(C):
        rows = np.concatenate([a[k * pc:(k + 1) * pc], b[k * pc:(k + 1) * pc]], 0)
        maps.append({"rows": np.ascontiguousarray(rows)})
    return maps


def combine(per_core_stats, cfg):
    B = cfg["B"]
    n = 2 * B
    d_sum = 0.0
    s_sum = 0.0
    for st in per_core_stats:
        st = np.asarray(st, dtype=np.float64)
        d_sum += st[:, 0].sum()
        s_sum += st[:, 1].sum()
    align = 2.0 - (2.0 / B) * d_sum
    uniform = (s_sum - n) / (n * (n - 1.0))
    return np.float32(align + uniform)


def run_spmd(a, b, cfg, trace=False):
    from concourse.bass_utils import run_bass_kernel_spmd

    nc = get_program(cfg)
    in_maps = shard_inputs(a, b, cfg)
    res = run_bass_kernel_spmd(
        nc, in_maps, core_ids=list(range(cfg["cores"])), trace=trace
    )
    out = combine([r["stats"] for r in res.results], cfg)
    return out, res


def kernel(embeddings_a, embeddings_b):
    out, _ = run_spmd(embeddings_a, embeddings_b, FULL_CFG, trace=False)
    return np.asarray(out)
